# revision 8
# baseline (speedup 1.0000x reference)
"""SMPL body-model (LBS) kernel for 8 Trainium2 NeuronCores.

Sharding: vertices split across the 8 cores (V=6890 -> 896/core padded);
batch (B=512) replicated on every core. Per core:
  verts[b,v,m] = sum_j w[v,j]*(A_t[b,j,m] + sum_c A_R[b,j,m,c]*vt[v,c]) + trans[b,m]
              + sum_c (sum_j w[v,j]*A_R[b,j,m,c]) * dvp[b,v,c]
with dvp = shape+pose blendshape deltas (one K=217 matmul against
[posefeat; betas]). Line 1 is a K=97 f32r matmul (pure PE). Line 2 needs an
elementwise product of two PE outputs (DVE, bf16), re-accumulated into the
verts PSUM tile via identity matmuls.
"""
import sys

sys.path.insert(0, '/opt/trn_rl_repo')

import contextlib

import numpy as np

import concourse.bass as bass
import concourse.mybir as mybir
import concourse.tile as tile
from concourse import bacc
from concourse.bass_utils import run_bass_kernel_spmd
from concourse.masks import make_identity

P = 128
B = 512
BT = B // P          # 4 batch tiles
J = 24
NB = 10
V = 6890
NCORES = 8
VL = 896             # vertices per core (padded)
VC = VL // P         # 7 vertex chunks per core
NPF = 207            # pose-feature length
KD = NPF + NB        # 217 = dvp contraction dim; rows = [pf(207); betas(10)]
KB = KD - P          # 89 = second K chunk
VPAD = 6912          # 54*128, padded V for the J-regressor matmul
KJ = VPAD // P       # 54 chunks

SMPL_PARENTS = [-1, 0, 0, 0, 1, 2, 3, 4, 5, 6, 7, 8, 9, 9, 9, 12, 13, 14,
                16, 17, 18, 19, 20, 21]
# FK groups: (child_lo, child_hi, parent_lo) with parent(c) = plo + (c - clo)
FK_GROUPS = [(1, 2, 0), (2, 3, 0), (3, 4, 0), (4, 7, 1), (7, 10, 4),
             (10, 12, 7), (12, 13, 9), (13, 14, 9), (14, 15, 9), (15, 16, 12),
             (16, 18, 13), (18, 20, 16), (20, 22, 18), (22, 24, 20)]
for _clo, _chi, _plo in FK_GROUPS:
    for _c in range(_clo, _chi):
        assert SMPL_PARENTS[_c] == _plo + (_c - _clo)

F32 = mybir.dt.float32
F32R = mybir.dt.float32r
BF16 = mybir.dt.bfloat16
MUL = mybir.AluOpType.mult
ADD = mybir.AluOpType.add
SUB = mybir.AluOpType.subtract

LAST_RESULTS = None  # for the local test harness


def build_kernel():
    nc = bacc.Bacc("TRN2", target_bir_lowering=False, debug=False,
                   num_devices=NCORES)

    # ---- DRAM I/O (per-core tensors; same program on all cores) ----
    d_pose = nc.dram_tensor("pose_r", [P, BT * J * 3], F32, kind="ExternalInput")
    d_betasT = nc.dram_tensor("betasT", [NB, B], F32R, kind="ExternalInput")
    d_transT = nc.dram_tensor("transT", [1, 3 * B], F32R, kind="ExternalInput")
    d_bigA = nc.dram_tensor("bigA", [P, 3 * VL], F32R, kind="ExternalInput")
    d_bigB = nc.dram_tensor("bigB", [KB, 3 * VL], F32R, kind="ExternalInput")
    d_wT = nc.dram_tensor("wTx", [J + 1, VL], F32R, kind="ExternalInput")
    d_vtT = nc.dram_tensor("vtT", [3, VL], F32R, kind="ExternalInput")
    d_jregT = nc.dram_tensor("jregT", [VPAD, J], F32R, kind="ExternalInput")
    d_sdvt = nc.dram_tensor("sdvt", [VPAD, 34], F32R, kind="ExternalInput")
    d_ipatA = nc.dram_tensor("ipatA", [P, 1], F32, kind="ExternalInput")
    d_ipatB = nc.dram_tensor("ipatB", [NPF - P, 1], F32, kind="ExternalInput")
    d_out = nc.dram_tensor("out_v", [VL, 3 * B], F32, kind="ExternalOutput")

    with tile.TileContext(nc) as tc, contextlib.ExitStack() as ctx:
        singles = ctx.enter_context(tc.tile_pool(name="singles", bufs=1))
        temps = ctx.enter_context(tc.tile_pool(name="temps", bufs=2))
        dram = ctx.enter_context(tc.tile_pool(name="drams", bufs=1, space="DRAM"))

        # ---------- persistent tiles ----------
        ident_f = singles.tile([P, P], F32)
        make_identity(nc, ident_f[:])
        ident_bf = singles.tile([P, P], BF16)
        make_identity(nc, ident_bf[:])

        bigA_sb = singles.tile([P, 3 * VL], F32R)
        nc.sync.dma_start(bigA_sb[:], d_bigA[:, :])
        bigB_sb = singles.tile([KB, 3 * VL], F32R)
        nc.sync.dma_start(bigB_sb[:], d_bigB[:, :])

        jreg_sb = singles.tile([P, KJ, J], F32R)
        nc.sync.dma_start(jreg_sb[:],
                          d_jregT[:, :].rearrange("(kc p) j -> p kc j", p=P))
        sdvt_sb = singles.tile([P, KJ, 34], F32R)
        nc.sync.dma_start(sdvt_sb[:],
                          d_sdvt[:, :].rearrange("(kc p) j -> p kc j", p=P))

        # Wbig [97, VL]: rows 0..23 w_T, 24..95 wvt[(c,j)] = w_T[j]*vt_T[c], 96 ones
        Wbig = singles.tile([97, VL], F32R)
        nc.sync.dma_start(Wbig[0:J, :], d_wT[0:J, :])
        nc.sync.dma_start(Wbig[96:97, :], d_wT[J:J + 1, :])
        wvt = temps.tile([72, VL], F32R, tag="wvt")
        for c in range(3):
            nc.sync.dma_start(wvt[c * J:(c + 1) * J, :], d_wT[0:J, :])
        vt_rep = temps.tile([72, VL], F32R, tag="vt_rep")
        nc.sync.dma_start(
            vt_rep[:],
            bass.AP(tensor=d_vtT.ap().tensor, offset=0,
                    ap=[[VL, 3], [0, J], [1, VL]]))
        nc.vector.tensor_tensor(wvt[:], wvt[:], vt_rep[:], MUL)
        nc.sync.dma_start(Wbig[J:J + 72, :], wvt[:])

        # phi: rows = [pf(207); betas(10)] split at 128
        phiA = singles.tile([P, B], F32R)       # pf rows 0..127
        phiB = singles.tile([KB, B], F32R)      # pf rows 128..206, betas 79..88
        nc.sync.dma_start(phiB[NPF - P:KB, :], d_betasT[:, :])
        betas_sb = singles.tile([NB, B], F32R)  # base-0 copy for the Jts matmul
        nc.sync.dma_start(betas_sb[:], d_betasT[:, :])

        pose_sb = singles.tile([P, BT, J, 3], F32)
        nc.sync.dma_start(pose_sb[:], d_pose[:, :].rearrange(
            "p (bt j c) -> p bt j c", bt=BT, j=J))

        iptA = singles.tile([P, 1], F32)
        nc.sync.dma_start(iptA[:], d_ipatA[:, :])
        iptB = singles.tile([NPF - P, 1], F32)
        nc.sync.dma_start(iptB[:], d_ipatB[:, :])

        # DRAM scratch
        jd_dram = dram.tile([J, 34], F32R)
        A_dram = dram.tile([J * 12, B], F32R)

        # ---------- early phase ----------
        with tc.tile_pool(name="psE", bufs=2, space="PSUM") as psE:
            # ===== J_dirs | J0 = J_reg @ [shapedirs | v_template] =====
            jd_ps = psE.tile([J, 34], F32, tag="jd")
            for kc in range(KJ):
                nc.tensor.matmul(jd_ps[:], jreg_sb[:, kc, :], sdvt_sb[:, kc, :],
                                 start=(kc == 0), stop=(kc == KJ - 1))
            jd_sb = temps.tile([J, 34], F32R, tag="jd_sb")
            nc.vector.tensor_copy(jd_sb[:], jd_ps[:])
            nc.sync.dma_start(jd_dram[:], jd_sb[:])
            Jd_r = singles.tile([NB, J, 3], F32R)   # [k, (j,c)]
            for c in range(3):
                nc.sync.dma_start(
                    Jd_r[:, :, c], bass.AP(tensor=jd_dram[:].tensor,
                                           offset=jd_dram[:].offset + c * NB,
                                           ap=[[1, NB], [34, J]]))
            J0_rep = singles.tile([P, J * 3], F32)   # J0 broadcast over partitions
            nc.gpsimd.dma_start(
                J0_rep[:], bass.AP(tensor=jd_dram[:].tensor,
                                   offset=jd_dram[:].offset + 30,
                                   ap=[[0, P], [34, J], [1, 3]]))

            # ===== Rodrigues (b-major) =====
            rod = ctx.enter_context(tc.tile_pool(name="rod", bufs=1))
            NJ = BT * J  # 96
            pr = pose_sb[:]
            sq = rod.tile([P, BT, J, 3], F32)
            nc.vector.tensor_tensor(sq[:], pr, pr, MUL)
            ss = rod.tile([P, NJ], F32)
            sqf = sq[:].rearrange("p bt j c -> p (bt j) c")
            nc.vector.tensor_tensor(ss[:], sqf[:, :, 0], sqf[:, :, 1], ADD)
            nc.vector.tensor_tensor(ss[:], ss[:], sqf[:, :, 2], ADD)
            eps_t = rod.tile([P, 1], F32)
            nc.vector.memset(eps_t[:], 1e-16)
            hpi_t = rod.tile([P, 1], F32)
            nc.vector.memset(hpi_t[:], float(np.pi / 2))
            ang = rod.tile([P, NJ], F32)
            nc.scalar.activation(ang[:], ss[:], mybir.ActivationFunctionType.Sqrt,
                                 bias=eps_t[:], scale=1.0)
            sin_t = rod.tile([P, NJ], F32)
            nc.scalar.activation(sin_t[:], ang[:], mybir.ActivationFunctionType.Sin)
            cos_t = rod.tile([P, NJ], F32)
            nc.scalar.activation(cos_t[:], ang[:], mybir.ActivationFunctionType.Sin,
                                 bias=hpi_t[:])
            inv = rod.tile([P, NJ], F32)
            nc.vector.reciprocal(inv[:], ang[:])
            axis = rod.tile([P, BT, J, 3], F32)
            invv = inv[:].rearrange("p (bt j) -> p bt j", bt=BT)
            nc.vector.tensor_tensor(axis[:], pr,
                                    invv[:, :, :, None].to_broadcast([P, BT, J, 3]),
                                    MUL)
            ones = rod.tile([P, NJ], F32)
            nc.vector.memset(ones[:], 1.0)
            omc = rod.tile([P, NJ], F32)
            nc.vector.tensor_tensor(omc[:], ones[:], cos_t[:], SUB)
            omcv = omc[:].rearrange("p (bt j) -> p bt j", bt=BT)
            sinv = sin_t[:].rearrange("p (bt j) -> p bt j", bt=BT)
            omc_a = rod.tile([P, BT, J, 3], F32)
            nc.vector.tensor_tensor(omc_a[:], axis[:],
                                    omcv[:, :, :, None].to_broadcast([P, BT, J, 3]),
                                    MUL)
            s_a = rod.tile([P, BT, J, 3], F32)
            nc.vector.tensor_tensor(s_a[:], axis[:],
                                    sinv[:, :, :, None].to_broadcast([P, BT, J, 3]),
                                    MUL)
            rot = singles.tile([P, BT, J, 9], F32)
            cosv = cos_t[:].rearrange("p (bt j) -> p bt j", bt=BT)
            tmp96 = rod.tile([P, BT, J], F32)
            rotv = rot[:].rearrange("p bt j (m n) -> p bt j m n", m=3)
            for m in range(3):
                nc.vector.tensor_tensor(tmp96[:], omc_a[:, :, :, m],
                                        axis[:, :, :, m], MUL)
                nc.vector.tensor_tensor(rotv[:, :, :, m, m], tmp96[:], cosv, ADD)
            KSIGN = {(0, 1): (2, -1), (0, 2): (1, 1), (1, 0): (2, 1),
                     (1, 2): (0, -1), (2, 0): (1, -1), (2, 1): (0, 1)}
            for (m, n), (k, sgn) in KSIGN.items():
                nc.vector.tensor_tensor(tmp96[:], omc_a[:, :, :, m],
                                        axis[:, :, :, n], MUL)
                nc.vector.tensor_tensor(rotv[:, :, :, m, n], tmp96[:],
                                        s_a[:, :, :, k], ADD if sgn > 0 else SUB)

            # ===== pose features -> phiA / phiB (PE transpose per b-tile) =====
            for bt in range(BT):
                pf_in = rot[:, bt, 1:, :].rearrange("p a b -> p (a b)")  # [128,207]
                ps_t = psE.tile([P, P], F32, tag="tpose")
                nc.tensor.transpose(ps_t[:], pf_in[:, 0:P], ident_f[:])
                nc.vector.tensor_copy(phiA[:, bt * P:(bt + 1) * P], ps_t[:])
                ps_t2 = psE.tile([P, P], F32, tag="tpose")
                nc.tensor.transpose(ps_t2[0:NPF - P, :], pf_in[:, P:NPF],
                                    ident_f[:])
                nc.vector.tensor_copy(phiB[0:NPF - P, bt * P:(bt + 1) * P],
                                      ps_t2[0:NPF - P, :])
            nc.vector.tensor_tensor(phiA[:, :], phiA[:, :],
                                    iptA[:, :].to_broadcast([P, B]), SUB)
            nc.vector.tensor_tensor(phiB[0:NPF - P, :], phiB[0:NPF - P, :],
                                    iptB[:, :].to_broadcast([NPF - P, B]), SUB)

            # ===== Jts (tiny PE) =====
            jts_sb = singles.tile([P, BT, J, 3], F32)
            for bt in range(BT):
                jts_ps = psE.tile([P, J * 3], F32, tag="tpose")
                nc.tensor.matmul(jts_ps[:], betas_sb[:, bt * P:(bt + 1) * P],
                                 Jd_r[:].rearrange("k j c -> k (j c)"),
                                 start=True, stop=True)
                nc.vector.tensor_tensor(
                    jts_sb[:, bt, :, :].rearrange("p j c -> p (j c)"),
                    jts_ps[:], J0_rep[:], ADD)

            # ===== T_local / FK / A (DVE, b-major) =====
            fk = ctx.enter_context(tc.tile_pool(name="fk", bufs=1))
            Tloc = fk.tile([P, BT, J, 3, 4], F32)
            Tw = fk.tile([P, BT, J, 3, 4], F32)
            rot5 = rot[:].rearrange("p bt j (m n) -> p bt j m n", m=3)
            for m in range(3):
                nc.vector.tensor_copy(Tloc[:, :, :, m, 0:3], rot5[:, :, :, m, :])
            nc.vector.tensor_copy(Tloc[:, :, 0, :, 3], jts_sb[:, :, 0, :])
            for clo, chi, plo in FK_GROUPS:
                g = chi - clo
                nc.vector.tensor_tensor(Tloc[:, :, clo:chi, :, 3],
                                        jts_sb[:, :, clo:chi, :],
                                        jts_sb[:, :, plo:plo + g, :], SUB)
            nc.vector.tensor_copy(Tw[:, :, 0, :, :], Tloc[:, :, 0, :, :])
            fk_tmp = fk.tile([P, BT, 3, 4], F32)
            for clo, chi, plo in FK_GROUPS:
                g = chi - clo
                for m in range(3):
                    out_m = Tw[:, :, clo:chi, m, :]          # [P, BT, g, 4]
                    tmp_m = fk_tmp[:, :, 0:g, :]
                    for k in range(3):
                        pk = Tw[:, :, plo:plo + g, m, k]     # [P, BT, g]
                        pk = pk[:, :, :, None].to_broadcast([P, BT, g, 4])
                        lk = Tloc[:, :, clo:chi, k, :]       # [P, BT, g, 4]
                        if k == 0:
                            nc.vector.tensor_tensor(out_m, pk, lk, MUL)
                        else:
                            nc.vector.tensor_tensor(tmp_m, pk, lk, MUL)
                            nc.vector.tensor_tensor(out_m, out_m, tmp_m, ADD)
                nc.vector.tensor_tensor(Tw[:, :, clo:chi, :, 3],
                                        Tw[:, :, clo:chi, :, 3],
                                        Tw[:, :, plo:plo + g, :, 3], ADD)
            # A adjust: t -= R @ Jts
            acc288 = fk.tile([P, BT, J, 3], F32)
            tmp288 = fk.tile([P, BT, J, 3], F32)
            for c in range(3):
                jc = jts_sb[:, :, :, c]
                jc = jc[:, :, :, None].to_broadcast([P, BT, J, 3])
                dst = acc288 if c == 0 else tmp288
                nc.vector.tensor_tensor(dst[:], Tw[:, :, :, :, c], jc, MUL)
                if c > 0:
                    nc.vector.tensor_tensor(acc288[:], acc288[:], tmp288[:], ADD)
            nc.vector.tensor_tensor(Tw[:, :, :, :, 3], Tw[:, :, :, :, 3],
                                    acc288[:], SUB)

            # ===== transpose A -> A_dram [288, 512] =====
            stage = temps.tile([P, 3, B], F32R, tag="Astage")
            for bt in range(BT):
                a_in = Tw[:, bt, :, :, :].rearrange("p a b c -> p (a b c)")
                for blk in range(3):
                    w = P if blk < 2 else 32
                    ps_t3 = psE.tile([P, P], F32, tag="tpose")
                    nc.tensor.transpose(ps_t3[0:w, :],
                                        a_in[:, blk * P: blk * P + w], ident_f[:])
                    nc.vector.tensor_copy(stage[0:w, blk, bt * P:(bt + 1) * P],
                                          ps_t3[0:w, :])
            for blk in range(3):
                w = P if blk < 2 else 32
                nc.sync.dma_start(A_dram[blk * P: blk * P + w, :],
                                  stage[0:w, blk, :])

        # ===== rearranged skinning operands =====
        TRrhs = singles.tile([J, 3, 3, B], F32R)     # [j, (m, c, b)]
        for m in range(3):
            for c in range(3):
                nc.sync.dma_start(TRrhs[:, m, c, :], A_dram[m * 4 + c::12, :])
        T1rhs = singles.tile([97, 3, B], F32R)       # [At(j); wvt(c,j); trans]
        for m in range(3):
            nc.sync.dma_start(T1rhs[0:J, m, :], A_dram[m * 4 + 3::12, :])
            for c in range(3):
                nc.sync.dma_start(T1rhs[J + c * J: J + (c + 1) * J, m, :],
                                  A_dram[m * 4 + c::12, :])
        nc.sync.dma_start(T1rhs[96:97, :, :],
                          d_transT[:, :].rearrange("o (m b) -> o m b", m=3))

        # ===== main vertex-chunk loop =====
        dvp_pool = ctx.enter_context(tc.tile_pool(name="dvp", bufs=2))
        tmp_pool = ctx.enter_context(tc.tile_pool(name="tmpmc", bufs=3))
        out_pool = ctx.enter_context(tc.tile_pool(name="outs", bufs=2))
        ps_dvp = ctx.enter_context(tc.tile_pool(name="psD", bufs=2, space="PSUM"))
        ps_tr = ctx.enter_context(tc.tile_pool(name="psT", bufs=2, space="PSUM"))
        ps_v = ctx.enter_context(tc.tile_pool(name="psV", bufs=1, space="PSUM"))

        for vc in range(VC):
            vsl = slice(vc * P, (vc + 1) * P)
            # dvp = [pf; betas] @ [pdT; sdT] -> bf16
            dvp_sb = dvp_pool.tile([P, 3, B], BF16, tag="dvp")
            for c in range(3):
                dps = ps_dvp.tile([P, B], F32, tag="dvpp")
                nc.tensor.matmul(dps[:],
                                 bigA_sb[:, c * VL + vc * P: c * VL + (vc + 1) * P],
                                 phiA[:], start=True, stop=False)
                nc.tensor.matmul(dps[:],
                                 bigB_sb[:, c * VL + vc * P: c * VL + (vc + 1) * P],
                                 phiB[:], start=False, stop=True)
                nc.scalar.copy(dvp_sb[:, c, :], dps[:])
            # term1 (template + translation) into the verts psum tile
            vps = ps_v.tile([P, 3, B], F32, tag="vps")
            for m in range(3):
                nc.tensor.matmul(vps[:, m, :], Wbig[:, vsl], T1rhs[:, m, :],
                                 start=True, stop=False)
            # term2: TR_mc (*) dvp_c accumulated via identity matmuls
            for c in range(3):
                for m in range(3):
                    trp = ps_tr.tile([P, B], F32, tag="trp")
                    nc.tensor.matmul(trp[:], Wbig[0:J, vsl], TRrhs[:, m, c, :],
                                     start=True, stop=True)
                    tmp_mc = tmp_pool.tile([P, B], BF16, tag="tmp_mc")
                    nc.vector.tensor_tensor(tmp_mc[:], trp[:], dvp_sb[:, c, :],
                                            MUL)
                    nc.tensor.matmul(vps[:, m, :], ident_bf[:], tmp_mc[:],
                                     start=False, stop=(c == 2))
            # egress
            vout = out_pool.tile([P, 3 * B], F32, tag="vout")
            nc.scalar.copy(vout[:], vps[:].rearrange("p m b -> p (m b)"))
            nc.sync.dma_start(d_out[vsl, :], vout[:])

    nc.compile()
    return nc


_NC_CACHE = None


def _get_nc():
    global _NC_CACHE
    if _NC_CACHE is None:
        _NC_CACHE = build_kernel()
    return _NC_CACHE


def kernel(pose, betas, trans, v_template, shapedirs, posedirs, J_regressor,
           weights, parents):
    global LAST_RESULTS
    pose = np.asarray(pose, np.float32)
    betas = np.asarray(betas, np.float32)
    trans = np.asarray(trans, np.float32)
    v_template = np.asarray(v_template, np.float32)
    shapedirs = np.asarray(shapedirs, np.float32)
    posedirs = np.asarray(posedirs, np.float32)
    J_regressor = np.asarray(J_regressor, np.float32)
    weights = np.asarray(weights, np.float32)

    # ---- host-side shard/layout prep ----
    pose_r = np.ascontiguousarray(
        pose.reshape(BT, P, J * 3).transpose(1, 0, 2).reshape(P, BT * J * 3))
    betasT = np.ascontiguousarray(betas.T)                      # [10, 512]
    transT = np.ascontiguousarray(trans.T.reshape(1, 3 * B))    # [1, (m,b)]

    VTOT = VL * NCORES
    sd_p = np.zeros((VTOT, 3, NB), np.float32); sd_p[:V] = shapedirs
    vt_p = np.zeros((VTOT, 3), np.float32); vt_p[:V] = v_template
    w_p = np.zeros((VTOT, J), np.float32); w_p[:V] = weights
    pd_p = np.zeros((NPF, VTOT, 3), np.float32)
    pd_p[:, :V, :] = posedirs.reshape(NPF, V, 3)

    jreg_p = np.zeros((VPAD, J), np.float32); jreg_p[:V] = J_regressor.T
    sdvt = np.zeros((VPAD, 34), np.float32)
    sdvt[:V, 0:30] = shapedirs.reshape(V, 30)   # col = c*10 + k
    sdvt[:V, 30:33] = v_template

    ipat = np.zeros((NPF, 1), np.float32)
    for r in range(NPF):
        if r % 9 in (0, 4, 8):
            ipat[r] = 1.0

    in_maps = []
    for core in range(NCORES):
        vsl = slice(core * VL, (core + 1) * VL)
        # bigrhs rows = [pdT(207); sdT(10)], cols = (c, v) c-major
        big = np.empty((KD, 3, VL), np.float32)
        big[0:NPF] = pd_p[:, vsl, :].transpose(0, 2, 1)   # [207, 3, VL]
        big[NPF:KD] = sd_p[vsl].transpose(2, 1, 0)        # [10, 3, VL]
        big = big.reshape(KD, 3 * VL)
        wTx = np.concatenate([w_p[vsl].T, np.ones((1, VL), np.float32)], axis=0)
        in_maps.append({
            "pose_r": pose_r,
            "betasT": betasT,
            "transT": transT,
            "bigA": np.ascontiguousarray(big[0:P]),
            "bigB": np.ascontiguousarray(big[P:KD]),
            "wTx": np.ascontiguousarray(wTx),
            "vtT": np.ascontiguousarray(vt_p[vsl].T),
            "jregT": jreg_p,
            "sdvt": sdvt,
            "ipatA": np.ascontiguousarray(ipat[0:P]),
            "ipatB": np.ascontiguousarray(ipat[P:NPF]),
        })

    nc = _get_nc()
    res = run_bass_kernel_spmd(nc, in_maps, core_ids=list(range(NCORES)))
    LAST_RESULTS = res

    verts = np.empty((B, V, 3), np.float32)
    for core in range(NCORES):
        lo = core * VL
        n = min(VL, V - lo)
        if n <= 0:
            break
        o = res.results[core]["out_v"].reshape(VL, 3, B)
        verts[:, lo:lo + n, :] = o[:n].transpose(2, 0, 1)
    return verts


if __name__ == "__main__":
    rng = np.random.default_rng(0)
    ins = dict(
        pose=rng.standard_normal((B, J * 3)).astype(np.float32) * 0.2,
        betas=rng.standard_normal((B, NB)).astype(np.float32),
        trans=rng.standard_normal((B, 3)).astype(np.float32) * 0.1,
        v_template=rng.standard_normal((V, 3)).astype(np.float32) * 0.5,
        shapedirs=rng.standard_normal((V, 3, NB)).astype(np.float32) * 0.01,
        posedirs=rng.standard_normal((NPF, V * 3)).astype(np.float32) * 0.01,
        J_regressor=np.abs(rng.standard_normal((J, V)).astype(np.float32)),
        weights=np.abs(rng.standard_normal((V, J)).astype(np.float32)),
        parents=np.array(SMPL_PARENTS, np.int32),
    )
    out = kernel(**ins)
    print("out", out.shape, out.dtype, np.abs(out).max())


# revision 11
# speedup vs baseline: 1.1548x; 1.1548x over previous
"""SMPL body-model (LBS) kernel for 8 Trainium2 NeuronCores.

Sharding: vertices split across the 8 cores (V=6890 -> 896/core padded);
batch (B=512) replicated on every core. Per core:
  verts[b,v,m] = sum_j w[v,j]*(A_t[b,j,m] + sum_c A_R[b,j,m,c]*vt[v,c]) + trans[b,m]
              + sum_c (sum_j w[v,j]*A_R[b,j,m,c]) * dvp[b,v,c]
with dvp = shape+pose blendshape deltas (one K=217 matmul against
[posefeat; betas]). Line 1 is a K=97 f32r matmul (pure PE). Line 2 needs an
elementwise product of two PE outputs (DVE, bf16), re-accumulated into the
verts PSUM tile via identity matmuls.
"""
import sys

sys.path.insert(0, '/opt/trn_rl_repo')

import contextlib

import ml_dtypes
import numpy as np

import concourse.bass as bass
import concourse.mybir as mybir
import concourse.tile as tile
from concourse import bacc
from concourse.bass_utils import run_bass_kernel_spmd
from concourse.masks import make_identity

P = 128
B = 512
BT = B // P          # 4 batch tiles
J = 24
NB = 10
V = 6890
NCORES = 8
VL = 896             # vertices per core (padded)
VC = VL // P         # 7 vertex chunks per core
NPF = 207            # pose-feature length
KD = NPF + NB        # 217 = dvp contraction dim; rows = [pf(207); betas(10)]
KB = KD - P          # 89 = second K chunk
VPAD = 6912          # 54*128, padded V for the J-regressor matmul
KJ = VPAD // P       # 54 chunks

SMPL_PARENTS = [-1, 0, 0, 0, 1, 2, 3, 4, 5, 6, 7, 8, 9, 9, 9, 12, 13, 14,
                16, 17, 18, 19, 20, 21]
# FK groups: (child_lo, child_hi, parent_lo) with parent(c) = plo + (c - clo)
FK_GROUPS = [(1, 2, 0), (2, 3, 0), (3, 4, 0), (4, 7, 1), (7, 10, 4),
             (10, 12, 7), (12, 13, 9), (13, 14, 9), (14, 15, 9), (15, 16, 12),
             (16, 18, 13), (18, 20, 16), (20, 22, 18), (22, 24, 20)]
for _clo, _chi, _plo in FK_GROUPS:
    for _c in range(_clo, _chi):
        assert SMPL_PARENTS[_c] == _plo + (_c - _clo)

F32 = mybir.dt.float32
F32R = mybir.dt.float32r
BF16 = mybir.dt.bfloat16
MUL = mybir.AluOpType.mult
ADD = mybir.AluOpType.add
SUB = mybir.AluOpType.subtract

LAST_RESULTS = None  # for the local test harness


def build_kernel():
    nc = bacc.Bacc("TRN2", target_bir_lowering=False, debug=False,
                   num_devices=NCORES)

    # ---- DRAM I/O (per-core tensors; same program on all cores) ----
    d_pose = nc.dram_tensor("pose_r", [P, BT * J * 3], F32, kind="ExternalInput")
    d_betasT = nc.dram_tensor("betasT", [NB, B], F32R, kind="ExternalInput")
    d_transT = nc.dram_tensor("transT", [1, 3 * B], F32R, kind="ExternalInput")
    d_bigA = nc.dram_tensor("bigA", [P, 3 * VL], BF16, kind="ExternalInput")
    d_bigB = nc.dram_tensor("bigB", [KB, 3 * VL], BF16, kind="ExternalInput")
    d_wT = nc.dram_tensor("wTx", [J + 1, VL], F32R, kind="ExternalInput")
    d_vtT = nc.dram_tensor("vtT", [3, VL], F32R, kind="ExternalInput")
    d_jregT = nc.dram_tensor("jregT", [P, KJ * J], F32R, kind="ExternalInput")
    d_sdvt = nc.dram_tensor("sdvt", [P, KJ * 34], F32R, kind="ExternalInput")
    d_ipatA = nc.dram_tensor("ipatA", [P, 1], F32, kind="ExternalInput")
    d_ipatB = nc.dram_tensor("ipatB", [NPF - P, 1], F32, kind="ExternalInput")
    d_wtbf = nc.dram_tensor("wTbf", [J, VL], BF16, kind="ExternalInput")
    d_out = nc.dram_tensor("out_v", [VL, 3 * B], F32, kind="ExternalOutput")

    with tile.TileContext(nc) as tc, contextlib.ExitStack() as ctx:
        singles = ctx.enter_context(tc.tile_pool(name="singles", bufs=1))
        temps = ctx.enter_context(tc.tile_pool(name="temps", bufs=2))
        dram = ctx.enter_context(tc.tile_pool(name="drams", bufs=1, space="DRAM"))

        # ---------- persistent tiles ----------
        ident_f = singles.tile([P, P], F32)
        make_identity(nc, ident_f[:])
        ident_bf = singles.tile([P, P], BF16)
        make_identity(nc, ident_bf[:])

        bigA_sb = singles.tile([P, 3 * VL], BF16)
        nc.sync.dma_start(bigA_sb[:], d_bigA[:, :])
        bigB_sb = singles.tile([KB, 3 * VL], BF16)
        nc.sync.dma_start(bigB_sb[:], d_bigB[:, :])

        jreg_sb = singles.tile([P, KJ, J], F32R)
        nc.sync.dma_start(jreg_sb[:],
                          d_jregT[:, :].rearrange("p (kc j) -> p kc j", j=J))
        sdvt_sb = singles.tile([P, KJ, 34], F32R)
        nc.sync.dma_start(sdvt_sb[:],
                          d_sdvt[:, :].rearrange("p (kc j) -> p kc j", j=34))

        # Wbig [97, VL]: rows 0..23 w_T, 24..95 wvt[(c,j)] = w_T[j]*vt_T[c], 96 ones
        Wbig = singles.tile([97, VL], F32R)
        nc.sync.dma_start(Wbig[0:J, :], d_wT[0:J, :])
        nc.sync.dma_start(Wbig[96:97, :], d_wT[J:J + 1, :])
        wvt = temps.tile([72, VL], F32R, tag="wvt")
        for c in range(3):
            nc.sync.dma_start(wvt[c * J:(c + 1) * J, :], d_wT[0:J, :])
        vt_rep = temps.tile([72, VL], F32R, tag="vt_rep")
        nc.sync.dma_start(
            vt_rep[:],
            bass.AP(tensor=d_vtT.ap().tensor, offset=0,
                    ap=[[VL, 3], [0, J], [1, VL]]))
        nc.vector.tensor_tensor(wvt[:], wvt[:], vt_rep[:], MUL)
        nc.sync.dma_start(Wbig[J:J + 72, :], wvt[:])

        # phi: rows = [pf(207); betas(10)] split at 128
        phiA = singles.tile([P, B], BF16)       # pf rows 0..127
        phiB = singles.tile([KB, B], BF16)      # pf rows 128..206, betas 79..88
        nc.gpsimd.dma_start(phiB[NPF - P:KB, :], d_betasT[:, :])
        wtr_bf = singles.tile([J, VL], BF16)
        nc.sync.dma_start(wtr_bf[:], d_wtbf[:, :])
        betas_sb = singles.tile([NB, B], F32R)  # base-0 copy for the Jts matmul
        nc.sync.dma_start(betas_sb[:], d_betasT[:, :])

        pose_sb = singles.tile([P, BT, J, 3], F32)
        nc.sync.dma_start(pose_sb[:], d_pose[:, :].rearrange(
            "p (bt j c) -> p bt j c", bt=BT, j=J))

        iptA = singles.tile([P, 1], F32)
        nc.sync.dma_start(iptA[:], d_ipatA[:, :])
        iptB = singles.tile([NPF - P, 1], F32)
        nc.sync.dma_start(iptB[:], d_ipatB[:, :])

        # DRAM scratch
        jd_dram = dram.tile([J, 34], F32R)
        A_dram = dram.tile([J * 12, B], F32R)

        # dvp pools opened first so their PSUM banks never alias the early
        # phase's -> the 42 dvp matmuls are free to run during the FK chain.
        dvp_pool = ctx.enter_context(tc.tile_pool(name="dvp", bufs=VC))
        ps_dvp = ctx.enter_context(tc.tile_pool(name="psD", bufs=2, space="PSUM"))

        # ---------- early phase ----------
        with tc.tile_pool(name="psE", bufs=2, space="PSUM") as psE:
            # ===== J_dirs | J0 = J_reg @ [shapedirs | v_template] =====
            jd_ps = psE.tile([J, 34], F32, tag="jd")
            for kc in range(KJ):
                nc.tensor.matmul(jd_ps[:], jreg_sb[:, kc, :], sdvt_sb[:, kc, :],
                                 start=(kc == 0), stop=(kc == KJ - 1))
            jd_sb = temps.tile([J, 34], F32R, tag="jd_sb")
            nc.vector.tensor_copy(jd_sb[:], jd_ps[:])
            nc.sync.dma_start(jd_dram[:], jd_sb[:])
            Jd_r = singles.tile([NB, J, 3], F32R)   # [k, (j,c)]
            for c in range(3):
                nc.sync.dma_start(
                    Jd_r[:, :, c], bass.AP(tensor=jd_dram[:].tensor,
                                           offset=jd_dram[:].offset + c * NB,
                                           ap=[[1, NB], [34, J]]))
            J0_rep = singles.tile([P, J * 3], F32)   # J0 broadcast over partitions
            nc.gpsimd.dma_start(
                J0_rep[:], bass.AP(tensor=jd_dram[:].tensor,
                                   offset=jd_dram[:].offset + 30,
                                   ap=[[0, P], [34, J], [1, 3]]))

            # ===== Rodrigues (b-major) =====
            rod = ctx.enter_context(tc.tile_pool(name="rod", bufs=1))
            NJ = BT * J  # 96
            pr = pose_sb[:]
            sq = rod.tile([P, BT, J, 3], F32)
            nc.vector.tensor_tensor(sq[:], pr, pr, MUL)
            ss = rod.tile([P, NJ], F32)
            sqf = sq[:].rearrange("p bt j c -> p (bt j) c")
            nc.vector.tensor_tensor(ss[:], sqf[:, :, 0], sqf[:, :, 1], ADD)
            nc.vector.tensor_tensor(ss[:], ss[:], sqf[:, :, 2], ADD)
            eps_t = rod.tile([P, 1], F32)
            nc.vector.memset(eps_t[:], 1e-16)
            hpi_t = rod.tile([P, 1], F32)
            nc.vector.memset(hpi_t[:], float(np.pi / 2))
            ang = rod.tile([P, NJ], F32)
            nc.scalar.activation(ang[:], ss[:], mybir.ActivationFunctionType.Sqrt,
                                 bias=eps_t[:], scale=1.0)
            sin_t = rod.tile([P, NJ], F32)
            nc.scalar.activation(sin_t[:], ang[:], mybir.ActivationFunctionType.Sin)
            cos_t = rod.tile([P, NJ], F32)
            nc.scalar.activation(cos_t[:], ang[:], mybir.ActivationFunctionType.Sin,
                                 bias=hpi_t[:])
            inv = rod.tile([P, NJ], F32)
            nc.vector.reciprocal(inv[:], ang[:])
            axis = rod.tile([P, BT, J, 3], F32)
            invv = inv[:].rearrange("p (bt j) -> p bt j", bt=BT)
            nc.vector.tensor_tensor(axis[:], pr,
                                    invv[:, :, :, None].to_broadcast([P, BT, J, 3]),
                                    MUL)
            ones = rod.tile([P, NJ], F32)
            nc.vector.memset(ones[:], 1.0)
            omc = rod.tile([P, NJ], F32)
            nc.vector.tensor_tensor(omc[:], ones[:], cos_t[:], SUB)
            omcv = omc[:].rearrange("p (bt j) -> p bt j", bt=BT)
            sinv = sin_t[:].rearrange("p (bt j) -> p bt j", bt=BT)
            omc_a = rod.tile([P, BT, J, 3], F32)
            nc.vector.tensor_tensor(omc_a[:], axis[:],
                                    omcv[:, :, :, None].to_broadcast([P, BT, J, 3]),
                                    MUL)
            s_a = rod.tile([P, BT, J, 3], F32)
            nc.vector.tensor_tensor(s_a[:], axis[:],
                                    sinv[:, :, :, None].to_broadcast([P, BT, J, 3]),
                                    MUL)
            rot = singles.tile([P, BT, J, 9], F32)
            cosv = cos_t[:].rearrange("p (bt j) -> p bt j", bt=BT)
            tmp96 = rod.tile([P, BT, J], F32)
            rotv = rot[:].rearrange("p bt j (m n) -> p bt j m n", m=3)
            for m in range(3):
                nc.vector.tensor_tensor(tmp96[:], omc_a[:, :, :, m],
                                        axis[:, :, :, m], MUL)
                nc.vector.tensor_tensor(rotv[:, :, :, m, m], tmp96[:], cosv, ADD)
            KSIGN = {(0, 1): (2, -1), (0, 2): (1, 1), (1, 0): (2, 1),
                     (1, 2): (0, -1), (2, 0): (1, -1), (2, 1): (0, 1)}
            for (m, n), (k, sgn) in KSIGN.items():
                nc.vector.tensor_tensor(tmp96[:], omc_a[:, :, :, m],
                                        axis[:, :, :, n], MUL)
                nc.vector.tensor_tensor(rotv[:, :, :, m, n], tmp96[:],
                                        s_a[:, :, :, k], ADD if sgn > 0 else SUB)

            # ===== pose features -> phiA / phiB (PE transpose per b-tile) =====
            for bt in range(BT):
                pf_in = rot[:, bt, 1:, :].rearrange("p a b -> p (a b)")  # [128,207]
                ps_t = psE.tile([P, P], F32, tag="tpose")
                nc.tensor.transpose(ps_t[:], pf_in[:, 0:P], ident_f[:])
                nc.vector.tensor_copy(phiA[:, bt * P:(bt + 1) * P], ps_t[:])
                ps_t2 = psE.tile([P, P], F32, tag="tpose")
                nc.tensor.transpose(ps_t2[0:NPF - P, :], pf_in[:, P:NPF],
                                    ident_f[:])
                nc.vector.tensor_copy(phiB[0:NPF - P, bt * P:(bt + 1) * P],
                                      ps_t2[0:NPF - P, :])
            nc.vector.tensor_tensor(phiA[:, :], phiA[:, :],
                                    iptA[:, :].to_broadcast([P, B]), SUB)
            nc.vector.tensor_tensor(phiB[0:NPF - P, :], phiB[0:NPF - P, :],
                                    iptB[:, :].to_broadcast([NPF - P, B]), SUB)

            # ===== dvp for every vertex chunk (PE, overlaps FK below) =====
            dvp_tiles = []
            for vc in range(VC):
                dvp_sb = dvp_pool.tile([P, 3, B], BF16, tag="dvp")
                for c in range(3):
                    dps = ps_dvp.tile([P, B], F32, tag="dvpp")
                    nc.tensor.matmul(
                        dps[:],
                        bigA_sb[:, c * VL + vc * P: c * VL + (vc + 1) * P],
                        phiA[:], start=True, stop=False)
                    nc.tensor.matmul(
                        dps[:],
                        bigB_sb[:, c * VL + vc * P: c * VL + (vc + 1) * P],
                        phiB[:], start=False, stop=True)
                    nc.scalar.copy(dvp_sb[:, c, :], dps[:])
                dvp_tiles.append(dvp_sb)

            # ===== Jts (tiny PE) =====
            jts_sb = singles.tile([P, BT, J, 3], F32)
            for bt in range(BT):
                jts_ps = psE.tile([P, J * 3], F32, tag="tpose")
                nc.tensor.matmul(jts_ps[:], betas_sb[:, bt * P:(bt + 1) * P],
                                 Jd_r[:].rearrange("k j c -> k (j c)"),
                                 start=True, stop=True)
                nc.vector.tensor_tensor(
                    jts_sb[:, bt, :, :].rearrange("p j c -> p (j c)"),
                    jts_ps[:], J0_rep[:], ADD)

            # ===== T_local / FK / A (DVE, b-major) =====
            fk = ctx.enter_context(tc.tile_pool(name="fk", bufs=1))
            Tloc = fk.tile([P, BT, J, 3, 4], F32)
            Tw = fk.tile([P, BT, J, 3, 4], F32)
            rot5 = rot[:].rearrange("p bt j (m n) -> p bt j m n", m=3)
            for m in range(3):
                nc.vector.tensor_copy(Tloc[:, :, :, m, 0:3], rot5[:, :, :, m, :])
            nc.vector.tensor_copy(Tloc[:, :, 0, :, 3], jts_sb[:, :, 0, :])
            for clo, chi, plo in FK_GROUPS:
                g = chi - clo
                nc.vector.tensor_tensor(Tloc[:, :, clo:chi, :, 3],
                                        jts_sb[:, :, clo:chi, :],
                                        jts_sb[:, :, plo:plo + g, :], SUB)
            nc.vector.tensor_copy(Tw[:, :, 0, :, :], Tloc[:, :, 0, :, :])
            fk_tmp = fk.tile([P, BT, 3, 4], F32)
            for clo, chi, plo in FK_GROUPS:
                g = chi - clo
                for m in range(3):
                    out_m = Tw[:, :, clo:chi, m, :]          # [P, BT, g, 4]
                    tmp_m = fk_tmp[:, :, 0:g, :]
                    for k in range(3):
                        pk = Tw[:, :, plo:plo + g, m, k]     # [P, BT, g]
                        pk = pk[:, :, :, None].to_broadcast([P, BT, g, 4])
                        lk = Tloc[:, :, clo:chi, k, :]       # [P, BT, g, 4]
                        if k == 0:
                            nc.vector.tensor_tensor(out_m, pk, lk, MUL)
                        else:
                            nc.vector.tensor_tensor(tmp_m, pk, lk, MUL)
                            nc.vector.tensor_tensor(out_m, out_m, tmp_m, ADD)
                nc.vector.tensor_tensor(Tw[:, :, clo:chi, :, 3],
                                        Tw[:, :, clo:chi, :, 3],
                                        Tw[:, :, plo:plo + g, :, 3], ADD)
            # A adjust: t -= R @ Jts
            acc288 = fk.tile([P, BT, J, 3], F32)
            tmp288 = fk.tile([P, BT, J, 3], F32)
            for c in range(3):
                jc = jts_sb[:, :, :, c]
                jc = jc[:, :, :, None].to_broadcast([P, BT, J, 3])
                dst = acc288 if c == 0 else tmp288
                nc.vector.tensor_tensor(dst[:], Tw[:, :, :, :, c], jc, MUL)
                if c > 0:
                    nc.vector.tensor_tensor(acc288[:], acc288[:], tmp288[:], ADD)
            nc.vector.tensor_tensor(Tw[:, :, :, :, 3], Tw[:, :, :, :, 3],
                                    acc288[:], SUB)

            # ===== transpose A -> A_dram [288, 512] =====
            stage = temps.tile([P, 3, B], F32R, tag="Astage")
            for bt in range(BT):
                a_in = Tw[:, bt, :, :, :].rearrange("p a b c -> p (a b c)")
                for blk in range(3):
                    w = P if blk < 2 else 32
                    ps_t3 = psE.tile([P, P], F32, tag="tpose")
                    nc.tensor.transpose(ps_t3[0:w, :],
                                        a_in[:, blk * P: blk * P + w], ident_f[:])
                    nc.vector.tensor_copy(stage[0:w, blk, bt * P:(bt + 1) * P],
                                          ps_t3[0:w, :])
            for blk in range(3):
                w = P if blk < 2 else 32
                nc.sync.dma_start(A_dram[blk * P: blk * P + w, :],
                                  stage[0:w, blk, :])

        # ===== rearranged skinning operands =====
        TRrhs = singles.tile([J, 3, 3, B], BF16)     # [j, (m, c, b)]
        for m in range(3):
            for c in range(3):
                nc.gpsimd.dma_start(TRrhs[:, m, c, :], A_dram[m * 4 + c::12, :])
        T1rhs = singles.tile([97, 3, B], F32R)       # [At(j); wvt(c,j); trans]
        for m in range(3):
            nc.sync.dma_start(T1rhs[0:J, m, :], A_dram[m * 4 + 3::12, :])
            for c in range(3):
                nc.sync.dma_start(T1rhs[J + c * J: J + (c + 1) * J, m, :],
                                  A_dram[m * 4 + c::12, :])
        nc.sync.dma_start(T1rhs[96:97, :, :],
                          d_transT[:, :].rearrange("o (m b) -> o m b", m=3))

        # ===== main vertex-chunk loop =====
        tmp_pool = ctx.enter_context(tc.tile_pool(name="tmpmc", bufs=4))
        out_pool = ctx.enter_context(tc.tile_pool(name="outs", bufs=2))
        ps_tr = ctx.enter_context(tc.tile_pool(name="psT", bufs=1, space="PSUM"))
        ps_v = ctx.enter_context(tc.tile_pool(name="psV", bufs=1, space="PSUM"))

        for vc in range(VC):
            vsl = slice(vc * P, (vc + 1) * P)
            dvp_sb = dvp_tiles[vc]
            # term1 (template + translation) into the verts psum tile
            vps = ps_v.tile([P, 3, B], F32, tag="vps")
            for m in range(3):
                nc.tensor.matmul(vps[:, m, :], Wbig[:, vsl], T1rhs[:, m, :],
                                 start=True, stop=False)
            # term2: TR_mc (*) dvp_c accumulated via identity matmuls
            for c in range(3):
                trc = ps_tr.tile([P, 3, B], F32, tag="trc")
                for m in range(3):
                    nc.tensor.matmul(trc[:, m, :], wtr_bf[:, vsl],
                                     TRrhs[:, m, c, :], start=True, stop=True)
                tmps = []
                for m in range(3):
                    tmp_mc = tmp_pool.tile([P, B], BF16, tag="tmp_mc")
                    nc.vector.tensor_tensor(tmp_mc[:], trc[:, m, :],
                                            dvp_sb[:, c, :], MUL)
                    tmps.append(tmp_mc)
                for m in range(3):
                    nc.tensor.matmul(vps[:, m, :], ident_bf[:], tmps[m][:],
                                     start=False, stop=(c == 2))
            # egress
            vout = out_pool.tile([P, 3 * B], F32, tag="vout")
            nc.scalar.copy(vout[:], vps[:].rearrange("p m b -> p (m b)"))
            nc.sync.dma_start(d_out[vsl, :], vout[:])

    nc.compile()
    return nc


_NC_CACHE = None


def _get_nc():
    global _NC_CACHE
    if _NC_CACHE is None:
        _NC_CACHE = build_kernel()
    return _NC_CACHE


def kernel(pose, betas, trans, v_template, shapedirs, posedirs, J_regressor,
           weights, parents):
    global LAST_RESULTS
    pose = np.asarray(pose, np.float32)
    betas = np.asarray(betas, np.float32)
    trans = np.asarray(trans, np.float32)
    v_template = np.asarray(v_template, np.float32)
    shapedirs = np.asarray(shapedirs, np.float32)
    posedirs = np.asarray(posedirs, np.float32)
    J_regressor = np.asarray(J_regressor, np.float32)
    weights = np.asarray(weights, np.float32)

    # ---- host-side shard/layout prep ----
    pose_r = np.ascontiguousarray(
        pose.reshape(BT, P, J * 3).transpose(1, 0, 2).reshape(P, BT * J * 3))
    betasT = np.ascontiguousarray(betas.T)                      # [10, 512]
    transT = np.ascontiguousarray(trans.T.reshape(1, 3 * B))    # [1, (m,b)]

    VTOT = VL * NCORES
    sd_p = np.zeros((VTOT, 3, NB), np.float32); sd_p[:V] = shapedirs
    vt_p = np.zeros((VTOT, 3), np.float32); vt_p[:V] = v_template
    w_p = np.zeros((VTOT, J), np.float32); w_p[:V] = weights
    pd_p = np.zeros((NPF, VTOT, 3), np.float32)
    pd_p[:, :V, :] = posedirs.reshape(NPF, V, 3)

    jreg_p = np.zeros((VPAD, J), np.float32); jreg_p[:V] = J_regressor.T
    sdvt = np.zeros((VPAD, 34), np.float32)
    sdvt[:V, 0:30] = shapedirs.reshape(V, 30)   # col = c*10 + k
    sdvt[:V, 30:33] = v_template
    # rearrange to [p, kc*J] so the device DMA is contiguous
    jreg_r = np.ascontiguousarray(
        jreg_p.reshape(KJ, P, J).transpose(1, 0, 2).reshape(P, KJ * J))
    sdvt_r = np.ascontiguousarray(
        sdvt.reshape(KJ, P, 34).transpose(1, 0, 2).reshape(P, KJ * 34))

    ipat = np.zeros((NPF, 1), np.float32)
    for r in range(NPF):
        if r % 9 in (0, 4, 8):
            ipat[r] = 1.0

    in_maps = []
    for core in range(NCORES):
        vsl = slice(core * VL, (core + 1) * VL)
        # bigrhs rows = [pdT(207); sdT(10)], cols = (c, v) c-major
        big = np.empty((KD, 3, VL), np.float32)
        big[0:NPF] = pd_p[:, vsl, :].transpose(0, 2, 1)   # [207, 3, VL]
        big[NPF:KD] = sd_p[vsl].transpose(2, 1, 0)        # [10, 3, VL]
        big = big.reshape(KD, 3 * VL)
        wTx = np.concatenate([w_p[vsl].T, np.ones((1, VL), np.float32)], axis=0)
        in_maps.append({
            "pose_r": pose_r,
            "betasT": betasT,
            "transT": transT,
            "bigA": np.ascontiguousarray(big[0:P]).astype(ml_dtypes.bfloat16),
            "bigB": np.ascontiguousarray(big[P:KD]).astype(ml_dtypes.bfloat16),
            "wTx": np.ascontiguousarray(wTx),
            "vtT": np.ascontiguousarray(vt_p[vsl].T),
            "jregT": jreg_r,
            "sdvt": sdvt_r,
            "ipatA": np.ascontiguousarray(ipat[0:P]),
            "ipatB": np.ascontiguousarray(ipat[P:NPF]),
            "wTbf": np.ascontiguousarray(w_p[vsl].T).astype(ml_dtypes.bfloat16),
        })

    nc = _get_nc()
    res = run_bass_kernel_spmd(nc, in_maps, core_ids=list(range(NCORES)))
    LAST_RESULTS = res

    verts = np.empty((B, V, 3), np.float32)
    for core in range(NCORES):
        lo = core * VL
        n = min(VL, V - lo)
        if n <= 0:
            break
        o = res.results[core]["out_v"].reshape(VL, 3, B)
        verts[:, lo:lo + n, :] = o[:n].transpose(2, 0, 1)
    return verts


if __name__ == "__main__":
    rng = np.random.default_rng(0)
    ins = dict(
        pose=rng.standard_normal((B, J * 3)).astype(np.float32) * 0.2,
        betas=rng.standard_normal((B, NB)).astype(np.float32),
        trans=rng.standard_normal((B, 3)).astype(np.float32) * 0.1,
        v_template=rng.standard_normal((V, 3)).astype(np.float32) * 0.5,
        shapedirs=rng.standard_normal((V, 3, NB)).astype(np.float32) * 0.01,
        posedirs=rng.standard_normal((NPF, V * 3)).astype(np.float32) * 0.01,
        J_regressor=np.abs(rng.standard_normal((J, V)).astype(np.float32)),
        weights=np.abs(rng.standard_normal((V, J)).astype(np.float32)),
        parents=np.array(SMPL_PARENTS, np.int32),
    )
    out = kernel(**ins)
    print("out", out.shape, out.dtype, np.abs(out).max())
